# revision 25
# baseline (speedup 1.0000x reference)
"""Trainium2 Bass kernel for nn_CategoryAlign_Module (pooling / cross Pearson).

Math (see reference):
  for each stream s in {1,2}:
    vec_b[k,c]  = sum_p preds[b,k,p] * feats[b,c,p] / sum_p preds[b,k,p]
    ctx_b[k,c]  = vec_b[k,c] / max(||vec_b[:,c]||_2, 1e-12)      (norm over K)
    ctx[k,c]    = mean_b ctx_b[k,c]
  out = pearson(ctx1, ctx2)   (center+normalize rows over C, then ctx1 @ ctx2^T)

Distribution: data-parallel over the batch dim, one batch element per
NeuronCore (B=8, 8 cores).  Each core computes its local normalized
context TRANSPOSED ([C on partitions, K in free] -- every element-wise
op runs 128-partition-parallel); the [128,38] payload per stream is
AllGather'd across the 8 cores (measured ~2x faster than AllReduce
here) and tree-summed on-chip.  Pearson is computed from the transposed
sums with the centering folded into the final matmul algebraically:
  out[i,j] = (G[i,j] - C*m0[i]*m1[j]) * ri0[i] * ri1[j],
  G = sum_c X0t[c,i] X1t[c,j],  m = colmean,  ri = 1/std.
(Pearson is invariant to the 1/B scale, so the mean's division is
skipped.)

Schedule:
  - the two streams are SERIALIZED in the DMA window: all of stream 0's
    bytes stream first, so stream 0's contraction, epilogue and
    AllGather complete UNDER stream 1's DMA window; only stream 1's
    AllGather (~12us) is exposed at the tail.
  - HWDGE queues are FIFO and triggers carry pacing waits that BLOCK
    the issuing engine, so per-engine instruction order is arranged so
    nothing latency-critical sits behind un-drained bulk: the stream-1
    collective staging/gather ride the queues only after bulk drains;
    stream 0's staging is a single small transfer slotted mid-stream.
  - epilogues are scalar-engine-free except one [128,2] Rsqrt (table
    pre-warmed); no single-partition [1,256] DVE ops anywhere (those
    cost 1.7us each -- DVE parallelism is across partitions).
  - a 4-byte warmup AllGather launched at kernel start absorbs the
    collective stack's one-time mesh init (~24us) under the bulk DMA.
"""

import sys

sys.path.insert(0, "/opt/trn_rl_repo")

import numpy as np

import concourse.bass as bass  # noqa: F401  (import order matters)
import concourse.bacc as bacc
import concourse.tile as tile
import concourse.mybir as mybir
from concourse import bass_utils, bass2jax  # noqa: F401

B, K, C, H, W = 8, 19, 256, 128, 128
P = H * W            # 16384 spatial positions
NCHUNK = P // 128    # 128 contraction chunks
CCW = C + 1          # channels + fused ones column (mask sums)
# per-stream contraction chunks in contiguous segments; even segments on
# the sync queue, odd on scalar (64 chunks each); small tail segments so
# the last arrival gates as little compute as possible
SEG_SIZES = (4,) * 32
assert sum(SEG_SIZES) == NCHUNK
N_CORES = 8
RC = 1.0 / np.sqrt(C)

F32 = mybir.dt.float32
BF16 = mybir.dt.bfloat16


def build_body(nc, tc, pret_d, ftrseg_d, identf_d, out_d, n_cores):
    add = mybir.AluOpType.add
    mult = mybir.AluOpType.mult
    rg = [list(range(n_cores))]
    AF = mybir.ActivationFunctionType

    with tc.tile_pool(name="persist", bufs=1) as PP, \
         tc.tile_pool(name="ft", bufs=1) as FTP, \
         tc.tile_pool(name="acc", bufs=1, space="PSUM") as PA, \
         tc.tile_pool(name="tail", bufs=1, space="PSUM") as TLP, \
         tc.tile_pool(name="dram", bufs=1, space="DRAM") as DP:

        ones19 = PP.tile([K, 1], F32, name="ones19")
        nc.vector.memset(ones19[:], 1.0)
        onesr19 = PP.tile([1, K], F32, name="onesr19")
        nc.vector.memset(onesr19[:], 1.0)
        onesr128 = PP.tile([1, 128], F32, name="onesr128")
        nc.vector.memset(onesr128[:], 1.0)
        onesc128 = PP.tile([128, 1], F32, name="onesc128")
        nc.vector.memset(onesc128[:], 1.0)

        id_f = PP.tile([K, K], F32, name="id_f")
        dw_in = DP.tile([1, 1], F32, name="dw_in")
        dw_out = DP.tile([n_cores, 1], F32, name="dw_out")
        PT = [PP.tile([128, NCHUNK * K], BF16, name=f"PT{s}")
              for s in (0, 1)]
        fseg = [[FTP.tile([128, nch * CCW], BF16, name=f"fs{s}{si}")
                 for si, nch in enumerate(SEG_SIZES)] for s in (0, 1)]
        psum_vec = [PA.tile([K, CCW], F32, name=f"pvec{s}") for s in (0, 1)]

        # single combined payload (bf16 over the wire): cols 0:38 stream
        # 0, 38:76 stream 1
        W2 = 2 * K
        ctxT = PP.tile([128, 2 * W2], BF16, name="ctxT")
        ag_in = DP.tile([128, 2 * W2], BF16, name="agin")
        ag_out = DP.tile([n_cores * 128, 2 * W2], BF16, name="agout")
        gg = PP.tile([128, n_cores * 2 * W2], BF16, name="gg")
        Xt = PP.tile([128, 2 * W2], F32, name="Xt")
        junk = PP.tile([128, K], F32, name="junk")

        qs = [nc.sync, nc.scalar]

        def seg_triggers(s, lo, hi):
            for si in range(lo, hi):
                qs[si % 2].dma_start(fseg[s][si][:], ftrseg_d[s][si][:])

        # ---- head: bulk triggers, stream 0 first; NOTHING that waits
        # on compute may sit between bulk triggers (it would stall the
        # engine and idle its FIFO queue) ----
        nc.sync.dma_start(PT[0][:], pret_d[0][:])
        nc.scalar.dma_start(dw_in[:], ones19[0:1, 0:1])
        nc.scalar.dma_start(id_f[:], identf_d[:])
        seg_triggers(0, 0, len(SEG_SIZES))
        nc.scalar.dma_start(PT[1][:], pret_d[1][:])
        seg_triggers(1, 0, len(SEG_SIZES))

        # warmup AllGather (gpsimd blocks on it; mesh init ~24us hidden)
        prev_cc = nc.gpsimd.collective_compute(
            "AllGather", mybir.AluOpType.bypass, replica_groups=rg,
            ins=[dw_in.opt()], outs=[dw_out.opt()])

        # pre-warm the scalar activation tables used later
        wa = PP.tile([1, 2], F32, name="wa")
        nc.scalar.sqrt(wa[:, 1:2], ones19[0:1, 0:1])
        nc.scalar.activation(wa[:, 0:1], ones19[0:1, 0:1], AF.Copy)

        def emit_matmuls(s, lo, hi):
            base = sum(SEG_SIZES[:lo])
            for si in range(lo, hi):
                t_ = fseg[s][si]
                for t in range(SEG_SIZES[si]):
                    i = base + t
                    nc.tensor.matmul(
                        psum_vec[s][:],
                        lhsT=PT[s][:, i * K:(i + 1) * K],
                        rhs=t_[:, t * CCW:(t + 1) * CCW],
                        start=(i == 0), stop=(i == NCHUNK - 1))
                base += SEG_SIZES[si]

        def emit_epilogue(s):
            """Transposed epilogue: writes the normalized local context
            into ctxT[:, s*2K:(s+1)*2K] ([c on partitions, (h,k) free]),
            everything partition-parallel."""
            # recip[k] = 1 / masksum[k], folded into the raw sums BEFORE
            # the transpose so no cross-partition broadcast is needed
            recip = PP.tile([K, 1], F32, name=f"recip{s}")
            nc.vector.reciprocal(recip[:], psum_vec[s][:, C:C + 1])
            vsb = PP.tile([K, C], F32, name=f"vsb{s}")
            nc.vector.tensor_scalar(vsb[:], psum_vec[s][:, 0:C],
                                    recip[:], None, op0=mult)
            # YR [128, 2K] = vec^T (stays in PSUM; DVE reads contiguous
            # PSUM at full rate)
            tpsY = TLP.tile([128, 2 * K], F32, name=f"tpsY{s}", tag="epi")
            for h in (0, 1):
                nc.tensor.matmul(
                    tpsY[:, h * K:(h + 1) * K],
                    lhsT=vsb[:, h * 128:(h + 1) * 128], rhs=id_f[:],
                    is_transpose=True, start=(h == 0), stop=(h == 1))
            # SBUF copy (DVE ops may read PSUM at most once per op)
            YR = PP.tile([128, 2 * K], F32, name=f"YR{s}")
            nc.vector.tensor_copy(YR[:], tpsY[:])
            # nsq[c,h] = sum_k vec^2
            nsq = PP.tile([128, 2], F32, name=f"nsq{s}")
            for h in (0, 1):
                nc.vector.scalar_tensor_tensor(
                    junk[:], YR[:, h * K:(h + 1) * K], onesc128[:],
                    YR[:, h * K:(h + 1) * K],
                    op0=mult, op1=mult, accum_out=nsq[:, h:h + 1])
            # rn = 1/||vec_col|| = sqrt(1/nsq)  (the reference's 1e-12
            # clamp is a no-op for non-degenerate input)
            nsi = PP.tile([128, 2], F32, name=f"nsi{s}")
            nc.vector.reciprocal(nsi[:], nsq[:])
            rn = PP.tile([128, 2], F32, name=f"rn{s}")
            nc.scalar.sqrt(rn[:], nsi[:])
            for h in (0, 1):
                nc.vector.tensor_scalar(ctxT[:, s * W2 + h * K:
                                             s * W2 + (h + 1) * K],
                                        YR[:, h * K:(h + 1) * K],
                                        rn[:, h:h + 1], None, op0=mult)

        # ---- contraction + epilogues (one combined AllGather later; no
        # compute-gated DMA may interleave with bulk triggers) ----
        emit_matmuls(0, 0, len(SEG_SIZES))
        # stream 1's first matmuls precede stream 0's epilogue PE ops so
        # the tensor engine doesn't stall on the epilogue's vector chain
        emit_matmuls(1, 0, 2)
        emit_epilogue(0)
        emit_matmuls(1, 2, len(SEG_SIZES))
        emit_epilogue(1)

        # ---- one AllGather of the combined [128, 76] payload ----
        nc.sync.dma_start(ag_in[:], ctxT[:])
        cc1 = nc.gpsimd.collective_compute(
            "AllGather", mybir.AluOpType.bypass, replica_groups=rg,
            ins=[ag_in.opt()], outs=[ag_out.opt()])
        bass._add_dep_helper(cc1.ins, prev_cc.ins, sync=False,
                             reason="collectives in program order")
        # bring the 8 blocks back as four quarter-gathers alternating
        # queues; pair-sums start as each quarter lands (vector+gpsimd)
        W4 = 2 * W2
        for j in range(4):
            qs[j % 2].dma_start(
                gg[:, j * 2 * W4:(j + 1) * 2 * W4].rearrange(
                    "r (b c) -> r b c", b=2),
                ag_out[j * 256:(j + 1) * 256, :].rearrange(
                    "(b r) c -> r b c", b=2))
        aa = [PP.tile([128, W4], F32, name=f"aa{j}") for j in range(4)]
        for j in range(4):
            nc.vector.tensor_tensor(
                aa[j][:], gg[:, 2 * j * W4:(2 * j + 1) * W4],
                gg[:, (2 * j + 1) * W4:(2 * j + 2) * W4], op=add)
        nc.vector.tensor_tensor(aa[0][:], aa[0][:], aa[1][:], op=add)
        nc.vector.tensor_tensor(aa[2][:], aa[2][:], aa[3][:], op=add)
        nc.vector.tensor_tensor(Xt[:], aa[0][:], aa[2][:], op=add)

        # ---- Pearson tail from transposed sums Xt ----
        # stats [19, 4] cols = (ms0, ssq0, ms1, ssq1)
        stats = TLP.tile([K, 4], F32, name="stats", tag="tl1")
        X2 = PP.tile([128, 2 * W2], F32, name="X2")
        nc.vector.tensor_mul(X2[:], Xt[:], Xt[:])
        for s in (0, 1):
            for h in (0, 1):
                nc.tensor.matmul(stats[:, 2 * s:2 * s + 1],
                                 lhsT=Xt[:, s * W2 + h * K:
                                         s * W2 + (h + 1) * K],
                                 rhs=onesc128[:],
                                 start=(h == 0), stop=(h == 1))
            for h in (0, 1):
                nc.tensor.matmul(stats[:, 2 * s + 1:2 * s + 2],
                                 lhsT=X2[:, s * W2 + h * K:
                                         s * W2 + (h + 1) * K],
                                 rhs=onesc128[:],
                                 start=(h == 0), stop=(h == 1))
        # per-k stats: u = ms/sqrt(C) (u1 negated for the rank-1 term),
        # var = ssq - ms^2/C, ri = 1/sqrt(var)
        kst = PP.tile([K, 4], F32, name="kst")  # cols u0, ri0, u1, ri1
        vv = PP.tile([K, 2], F32, name="vv")
        ww = PP.tile([K, 2], F32, name="ww")
        for s in (0, 1):
            nc.vector.tensor_scalar(ww[:, s:s + 1], stats[:, 2 * s:2 * s + 1],
                                    stats[:, 2 * s:2 * s + 1], 1.0 / C,
                                    op0=mult, op1=mult)
            nc.vector.tensor_tensor(vv[:, s:s + 1],
                                    stats[:, 2 * s + 1:2 * s + 2],
                                    ww[:, s:s + 1],
                                    op=mybir.AluOpType.subtract)
            nc.vector.tensor_scalar_mul(kst[:, 2 * s:2 * s + 1],
                                        stats[:, 2 * s:2 * s + 1],
                                        RC if s == 0 else -RC)
        sd = PP.tile([K, 2], F32, name="sd")
        nc.scalar.sqrt(sd[:], vv[:])
        for s in (0, 1):
            nc.vector.reciprocal(kst[:, 2 * s + 1:2 * s + 2], sd[:, s:s + 1])
        # transpose (u0, ri0, u1, ri1) columns to rows of kT [1, 4*K]
        kT = TLP.tile([1, 4 * K], F32, name="kT", tag="tl2")
        for j in range(4):
            nc.tensor.matmul(kT[:, j * K:(j + 1) * K],
                             lhsT=kst[:, j:j + 1], rhs=id_f[:],
                             is_transpose=True,
                             start=(j == 0), stop=(j == 3))
        kT_sb = PP.tile([1, 4 * K], F32, name="kT_sb")
        nc.scalar.copy(kT_sb[:], kT[:])
        # po = G - (C m0) (m1)^T  (u1 pre-negated)
        po = TLP.tile([K, K], F32, name="po", tag="tl3")
        for h in (0, 1):
            nc.tensor.matmul(po[:], lhsT=Xt[:, h * K:(h + 1) * K],
                             rhs=Xt[:, W2 + h * K:W2 + (h + 1) * K],
                             start=(h == 0), stop=False)
        nc.tensor.matmul(po[:], lhsT=kT_sb[:, 0:K], rhs=kT_sb[:, 2 * K:3 * K],
                         start=False, stop=True)
        # broadcast ri1 across partitions, then out = po * ri0 * ri1
        bci = TLP.tile([K, K], F32, name="bci", tag="tl4")
        nc.tensor.matmul(bci[:], lhsT=onesr19[:], rhs=kT_sb[:, 3 * K:4 * K],
                         start=True, stop=True)
        bci_sb = PP.tile([K, K], F32, name="bci_sb")
        nc.vector.tensor_copy(bci_sb[:], bci[:])
        osb = PP.tile([K, K], F32, name="osb")
        nc.vector.scalar_tensor_tensor(osb[:], po[:], kst[:, 1:2], bci_sb[:],
                                       op0=mult, op1=mult)
        nc.sync.dma_start(out_d[:], osb[:])


def build(n_cores=N_CORES):
    nc = bacc.Bacc("TRN2", target_bir_lowering=False, debug=False,
                   enable_asserts=False, num_devices=n_cores)
    pret_d = [nc.dram_tensor(f"pret{s}", [128, NCHUNK * K], BF16,
                             kind="ExternalInput").ap() for s in (1, 2)]
    ftrseg_d = [
        [nc.dram_tensor(f"ftr{s + 1}s{si}", [128, nch * CCW], BF16,
                        kind="ExternalInput").ap()
         for si, nch in enumerate(SEG_SIZES)]
        for s in (0, 1)]
    identf_d = nc.dram_tensor("identf", [K, K], F32, kind="ExternalInput").ap()
    out_d = nc.dram_tensor("out", [K, K], F32, kind="ExternalOutput").ap()
    with tile.TileContext(nc) as tc:
        build_body(nc, tc, pret_d, ftrseg_d, identf_d, out_d, n_cores)
    nc.compile()
    return nc


_NC_CACHE = {}


def _get_nc():
    if "nc" not in _NC_CACHE:
        _NC_CACHE["nc"] = build(N_CORES)
    return _NC_CACHE["nc"]


class Runner:
    """Executes the compiled Bass program on the first `n_cores` jax
    devices via shard_map, with inputs pre-staged on the devices (the
    analog of the native path's input pre-load in run_neff) so all
    cores start the NEFF near-simultaneously."""

    def __init__(self, nc, n_cores):
        import jax
        from jax.experimental.shard_map import shard_map
        from jax.sharding import Mesh, PartitionSpec, NamedSharding

        bass2jax.install_neuronx_cc_hook()
        self.jax = jax
        self.nc = nc
        self.n_cores = n_cores
        assert nc.dbg_addr is None
        partition_name = (nc.partition_id_tensor.name
                          if nc.partition_id_tensor else None)
        in_names, out_names, out_avals = [], [], []
        for alloc in nc.m.functions[0].allocations:
            if not isinstance(alloc, mybir.MemoryLocationSet):
                continue
            name = alloc.memorylocations[0].name
            if alloc.kind == "ExternalInput":
                if name != partition_name:
                    in_names.append(name)
            elif alloc.kind == "ExternalOutput":
                shape = tuple(alloc.tensor_shape)
                dtype = mybir.dt.np(alloc.dtype)
                out_names.append(name)
                out_avals.append(jax.core.ShapedArray(shape, dtype))
        self.param_names = list(in_names)
        n_params = len(in_names)
        full_in_names = list(in_names) + list(out_names)
        if partition_name is not None:
            full_in_names.append(partition_name)
        full_in_names = tuple(full_in_names)
        donate = tuple(range(n_params, n_params + len(out_names)))
        self.out_names = out_names
        self.out_avals = out_avals

        def _body(*args):
            operands = list(args)
            if partition_name is not None:
                operands.append(bass2jax.partition_id_tensor())
            outs = bass2jax._bass_exec_p.bind(
                *operands,
                out_avals=tuple(out_avals),
                in_names=full_in_names,
                out_names=tuple(out_names),
                lowering_input_output_aliases=(),
                sim_require_finite=True,
                sim_require_nnan=True,
                nc=nc,
            )
            return tuple(outs)

        devices = jax.devices()[:n_cores]
        assert len(devices) == n_cores
        self.mesh = Mesh(np.asarray(devices), ("core",))
        in_specs = (PartitionSpec("core"),) * (n_params + len(out_names))
        out_specs = (PartitionSpec("core"),) * len(out_names)
        self.fn = jax.jit(
            shard_map(_body, mesh=self.mesh, in_specs=in_specs,
                      out_specs=out_specs, check_rep=False),
            donate_argnums=donate, keep_unused=True)
        self.sharding = NamedSharding(self.mesh, PartitionSpec("core"))

    def put(self, in_maps):
        concat = [
            np.concatenate([np.asarray(in_maps[c][n])
                            for c in range(self.n_cores)], axis=0)
            for n in self.param_names
        ]
        arrs = [self.jax.device_put(a, self.sharding) for a in concat]
        self.jax.block_until_ready(arrs)
        return arrs

    def zeros(self):
        zs = [self.jax.device_put(
            np.zeros((self.n_cores * a.shape[0], *a.shape[1:]), a.dtype),
            self.sharding) for a in self.out_avals]
        self.jax.block_until_ready(zs)
        return zs

    def exec(self, dev_in):
        outs = self.fn(*dev_in, *self.zeros())
        self.jax.block_until_ready(outs)
        return {
            name: np.asarray(outs[i]).reshape(
                self.n_cores, *self.out_avals[i].shape)
            for i, name in enumerate(self.out_names)
        }


def _get_runner():
    if "runner" not in _NC_CACHE:
        _NC_CACHE["runner"] = Runner(_get_nc(), N_CORES)
    return _NC_CACHE["runner"]


def make_in_maps(preds1, feats1, preds2, feats2):
    import ml_dtypes
    bf16 = ml_dtypes.bfloat16
    identf = np.eye(K, dtype=np.float32)
    per_stream = {}
    for s, (preds, feats) in enumerate(
            ((preds1, feats1), (preds2, feats2)), start=1):
        # preds [B,K,H,W] -> [B, W(v), H(u), K] -> [B, 128, 128*19]:
        # chunk u's columns are P^T[u*128:(u+1)*128, :19] with the
        # spatial index on partitions
        pr = np.ascontiguousarray(
            preds.astype(bf16).transpose(0, 3, 2, 1)
        ).reshape(B, 128, NCHUNK * K)
        # feats [B,C,H,W] -> [B, W, H, C (+ ones)] -> [B, 128, 128*257]:
        # chunk u is the [w, c] block at h=u, matching pret's chunking;
        # the fused ones column makes psum[:, 256] the mask sums
        ft = np.empty((B, W, H, CCW), dtype=bf16)
        ft[..., :C] = feats.astype(bf16).transpose(0, 3, 2, 1)
        ft[..., C] = 1.0
        per_stream[s] = (pr, ft.reshape(B, 128, NCHUNK * CCW))
    in_maps = []
    for b in range(B):
        m = {
            "pret1": per_stream[1][0][b],
            "pret2": per_stream[2][0][b],
            "identf": identf,
        }
        for s in (1, 2):
            base = 0
            for si, nch in enumerate(SEG_SIZES):
                m[f"ftr{s}s{si}"] = np.ascontiguousarray(
                    per_stream[s][1][b][:, base * CCW:(base + nch) * CCW])
                base += nch
        in_maps.append(m)
    return in_maps


def kernel(preds1, feats1, preds2, feats2):
    runner = _get_runner()
    in_maps = make_in_maps(preds1, feats1, preds2, feats2)
    dev_in = runner.put(in_maps)
    outs = runner.exec(dev_in)
    return np.asarray(outs["out"][0], dtype=np.float32)


# revision 26
# speedup vs baseline: 1.0080x; 1.0080x over previous
"""Trainium2 Bass kernel for nn_CategoryAlign_Module (pooling / cross Pearson).

Math (see reference):
  for each stream s in {1,2}:
    vec_b[k,c]  = sum_p preds[b,k,p] * feats[b,c,p] / sum_p preds[b,k,p]
    ctx_b[k,c]  = vec_b[k,c] / max(||vec_b[:,c]||_2, 1e-12)      (norm over K)
    ctx[k,c]    = mean_b ctx_b[k,c]
  out = pearson(ctx1, ctx2)   (center+normalize rows over C, then ctx1 @ ctx2^T)

Distribution: data-parallel over the batch dim, one batch element per
NeuronCore (B=8, 8 cores).  Each core computes its local normalized
context TRANSPOSED ([C on partitions, K in free] -- every element-wise
op runs 128-partition-parallel); the [128,38] payload per stream is
AllGather'd across the 8 cores (measured ~2x faster than AllReduce
here) and tree-summed on-chip.  Pearson is computed from the transposed
sums with the centering folded into the final matmul algebraically:
  out[i,j] = (G[i,j] - C*m0[i]*m1[j]) * ri0[i] * ri1[j],
  G = sum_c X0t[c,i] X1t[c,j],  m = colmean,  ri = 1/std.
(Pearson is invariant to the 1/B scale, so the mean's division is
skipped.)

Schedule:
  - the two streams are SERIALIZED in the DMA window: all of stream 0's
    bytes stream first, so stream 0's contraction, epilogue and
    AllGather complete UNDER stream 1's DMA window; only stream 1's
    AllGather (~12us) is exposed at the tail.
  - HWDGE queues are FIFO and triggers carry pacing waits that BLOCK
    the issuing engine, so per-engine instruction order is arranged so
    nothing latency-critical sits behind un-drained bulk: the stream-1
    collective staging/gather ride the queues only after bulk drains;
    stream 0's staging is a single small transfer slotted mid-stream.
  - epilogues are scalar-engine-free except one [128,2] Rsqrt (table
    pre-warmed); no single-partition [1,256] DVE ops anywhere (those
    cost 1.7us each -- DVE parallelism is across partitions).
  - a 4-byte warmup AllGather launched at kernel start absorbs the
    collective stack's one-time mesh init (~24us) under the bulk DMA.
"""

import sys

sys.path.insert(0, "/opt/trn_rl_repo")

import numpy as np

import concourse.bass as bass  # noqa: F401  (import order matters)
import concourse.bacc as bacc
import concourse.tile as tile
import concourse.mybir as mybir
from concourse import bass_utils, bass2jax  # noqa: F401

B, K, C, H, W = 8, 19, 256, 128, 128
P = H * W            # 16384 spatial positions
NCHUNK = P // 128    # 128 contraction chunks
CCW = C + 1          # channels + fused ones column (mask sums)
# per-stream contraction chunks in contiguous segments; even segments on
# the sync queue, odd on scalar (64 chunks each); small tail segments so
# the last arrival gates as little compute as possible
SEG_SIZES = (8,) * 16
assert sum(SEG_SIZES) == NCHUNK
N_CORES = 8
RC = 1.0 / np.sqrt(C)

F32 = mybir.dt.float32
BF16 = mybir.dt.bfloat16


def build_body(nc, tc, pret_d, ftrseg_d, identf_d, out_d, n_cores):
    add = mybir.AluOpType.add
    mult = mybir.AluOpType.mult
    rg = [list(range(n_cores))]
    AF = mybir.ActivationFunctionType

    with tc.tile_pool(name="persist", bufs=1) as PP, \
         tc.tile_pool(name="ft", bufs=1) as FTP, \
         tc.tile_pool(name="acc", bufs=1, space="PSUM") as PA, \
         tc.tile_pool(name="tail", bufs=1, space="PSUM") as TLP, \
         tc.tile_pool(name="dram", bufs=1, space="DRAM") as DP:

        ones19 = PP.tile([K, 1], F32, name="ones19")
        nc.vector.memset(ones19[:], 1.0)
        onesr19 = PP.tile([1, K], F32, name="onesr19")
        nc.vector.memset(onesr19[:], 1.0)
        onesr128 = PP.tile([1, 128], F32, name="onesr128")
        nc.vector.memset(onesr128[:], 1.0)
        onesc128 = PP.tile([128, 1], F32, name="onesc128")
        nc.vector.memset(onesc128[:], 1.0)

        id_f = PP.tile([K, K], F32, name="id_f")
        dw_in = DP.tile([1, 1], F32, name="dw_in")
        dw_out = DP.tile([n_cores, 1], F32, name="dw_out")
        PT = [PP.tile([128, NCHUNK * K], BF16, name=f"PT{s}")
              for s in (0, 1)]
        fseg = [[FTP.tile([128, nch * CCW], BF16, name=f"fs{s}{si}")
                 for si, nch in enumerate(SEG_SIZES)] for s in (0, 1)]
        psum_vec = [PA.tile([K, CCW], F32, name=f"pvec{s}") for s in (0, 1)]

        # single combined payload (bf16 over the wire): cols 0:38 stream
        # 0, 38:76 stream 1
        W2 = 2 * K
        ctxT = PP.tile([128, 2 * W2], BF16, name="ctxT")
        ag_in = DP.tile([128, 2 * W2], BF16, name="agin")
        ag_out = DP.tile([n_cores * 128, 2 * W2], BF16, name="agout")
        gg = PP.tile([128, n_cores * 2 * W2], BF16, name="gg")
        Xt = PP.tile([128, 2 * W2], F32, name="Xt")
        junk = PP.tile([128, K], F32, name="junk")

        qs = [nc.sync, nc.scalar]

        def seg_triggers(s, lo, hi):
            for si in range(lo, hi):
                qs[si % 2].dma_start(fseg[s][si][:], ftrseg_d[s][si][:])

        # ---- head: bulk triggers, stream 0 first; NOTHING that waits
        # on compute may sit between bulk triggers (it would stall the
        # engine and idle its FIFO queue) ----
        nc.sync.dma_start(PT[0][:], pret_d[0][:])
        nc.scalar.dma_start(dw_in[:], ones19[0:1, 0:1])
        nc.scalar.dma_start(id_f[:], identf_d[:])
        seg_triggers(0, 0, len(SEG_SIZES))
        nc.scalar.dma_start(PT[1][:], pret_d[1][:])
        seg_triggers(1, 0, len(SEG_SIZES))

        # warmup AllGather (gpsimd blocks on it; mesh init ~24us hidden)
        prev_cc = nc.gpsimd.collective_compute(
            "AllGather", mybir.AluOpType.bypass, replica_groups=rg,
            ins=[dw_in.opt()], outs=[dw_out.opt()])

        # pre-warm the scalar activation tables used later
        wa = PP.tile([1, 2], F32, name="wa")
        nc.scalar.sqrt(wa[:, 1:2], ones19[0:1, 0:1])
        nc.scalar.activation(wa[:, 0:1], ones19[0:1, 0:1], AF.Copy)

        def emit_matmuls(s, lo, hi):
            base = sum(SEG_SIZES[:lo])
            for si in range(lo, hi):
                t_ = fseg[s][si]
                for t in range(SEG_SIZES[si]):
                    i = base + t
                    nc.tensor.matmul(
                        psum_vec[s][:],
                        lhsT=PT[s][:, i * K:(i + 1) * K],
                        rhs=t_[:, t * CCW:(t + 1) * CCW],
                        start=(i == 0), stop=(i == NCHUNK - 1))
                base += SEG_SIZES[si]

        def emit_epilogue(s):
            """Transposed epilogue: writes the normalized local context
            into ctxT[:, s*2K:(s+1)*2K] ([c on partitions, (h,k) free]),
            everything partition-parallel."""
            # recip[k] = 1 / masksum[k], folded into the raw sums BEFORE
            # the transpose so no cross-partition broadcast is needed
            recip = PP.tile([K, 1], F32, name=f"recip{s}")
            nc.vector.reciprocal(recip[:], psum_vec[s][:, C:C + 1])
            vsb = PP.tile([K, C], F32, name=f"vsb{s}")
            nc.vector.tensor_scalar(vsb[:], psum_vec[s][:, 0:C],
                                    recip[:], None, op0=mult)
            # YR [128, 2K] = vec^T (stays in PSUM; DVE reads contiguous
            # PSUM at full rate)
            tpsY = TLP.tile([128, 2 * K], F32, name=f"tpsY{s}", tag="epi")
            for h in (0, 1):
                nc.tensor.matmul(
                    tpsY[:, h * K:(h + 1) * K],
                    lhsT=vsb[:, h * 128:(h + 1) * 128], rhs=id_f[:],
                    is_transpose=True, start=(h == 0), stop=(h == 1))
            # SBUF copy (DVE ops may read PSUM at most once per op)
            YR = PP.tile([128, 2 * K], F32, name=f"YR{s}")
            nc.vector.tensor_copy(YR[:], tpsY[:])
            # nsq[c,h] = sum_k vec^2
            nsq = PP.tile([128, 2], F32, name=f"nsq{s}")
            for h in (0, 1):
                nc.vector.scalar_tensor_tensor(
                    junk[:], YR[:, h * K:(h + 1) * K], onesc128[:],
                    YR[:, h * K:(h + 1) * K],
                    op0=mult, op1=mult, accum_out=nsq[:, h:h + 1])
            # rn = 1/||vec_col|| = sqrt(1/nsq)  (the reference's 1e-12
            # clamp is a no-op for non-degenerate input)
            nsi = PP.tile([128, 2], F32, name=f"nsi{s}")
            nc.vector.reciprocal(nsi[:], nsq[:])
            rn = PP.tile([128, 2], F32, name=f"rn{s}")
            nc.scalar.sqrt(rn[:], nsi[:])
            for h in (0, 1):
                nc.vector.tensor_scalar(ctxT[:, s * W2 + h * K:
                                             s * W2 + (h + 1) * K],
                                        YR[:, h * K:(h + 1) * K],
                                        rn[:, h:h + 1], None, op0=mult)

        # ---- contraction + epilogues (one combined AllGather later; no
        # compute-gated DMA may interleave with bulk triggers) ----
        emit_matmuls(0, 0, len(SEG_SIZES))
        # stream 1's first matmuls precede stream 0's epilogue PE ops so
        # the tensor engine doesn't stall on the epilogue's vector chain
        emit_matmuls(1, 0, 2)
        emit_epilogue(0)
        emit_matmuls(1, 2, len(SEG_SIZES))
        emit_epilogue(1)

        # ---- one AllGather of the combined [128, 76] payload ----
        nc.sync.dma_start(ag_in[:], ctxT[:])
        cc1 = nc.gpsimd.collective_compute(
            "AllGather", mybir.AluOpType.bypass, replica_groups=rg,
            ins=[ag_in.opt()], outs=[ag_out.opt()])
        bass._add_dep_helper(cc1.ins, prev_cc.ins, sync=False,
                             reason="collectives in program order")
        # bring the 8 blocks back as four quarter-gathers alternating
        # queues; pair-sums start as each quarter lands (vector+gpsimd)
        W4 = 2 * W2
        for j in range(4):
            qs[j % 2].dma_start(
                gg[:, j * 2 * W4:(j + 1) * 2 * W4].rearrange(
                    "r (b c) -> r b c", b=2),
                ag_out[j * 256:(j + 1) * 256, :].rearrange(
                    "(b r) c -> r b c", b=2))
        aa = [PP.tile([128, W4], F32, name=f"aa{j}") for j in range(4)]
        for j in range(4):
            nc.vector.tensor_tensor(
                aa[j][:], gg[:, 2 * j * W4:(2 * j + 1) * W4],
                gg[:, (2 * j + 1) * W4:(2 * j + 2) * W4], op=add)
        nc.vector.tensor_tensor(aa[0][:], aa[0][:], aa[1][:], op=add)
        nc.vector.tensor_tensor(aa[2][:], aa[2][:], aa[3][:], op=add)
        nc.vector.tensor_tensor(Xt[:], aa[0][:], aa[2][:], op=add)

        # ---- Pearson tail from transposed sums Xt ----
        # stats [19, 4] cols = (ms0, ssq0, ms1, ssq1)
        stats = TLP.tile([K, 4], F32, name="stats", tag="tl1")
        X2 = PP.tile([128, 2 * W2], F32, name="X2")
        nc.vector.tensor_mul(X2[:], Xt[:], Xt[:])
        for s in (0, 1):
            for h in (0, 1):
                nc.tensor.matmul(stats[:, 2 * s:2 * s + 1],
                                 lhsT=Xt[:, s * W2 + h * K:
                                         s * W2 + (h + 1) * K],
                                 rhs=onesc128[:],
                                 start=(h == 0), stop=(h == 1))
            for h in (0, 1):
                nc.tensor.matmul(stats[:, 2 * s + 1:2 * s + 2],
                                 lhsT=X2[:, s * W2 + h * K:
                                         s * W2 + (h + 1) * K],
                                 rhs=onesc128[:],
                                 start=(h == 0), stop=(h == 1))
        # per-k stats: u = ms/sqrt(C) (u1 negated for the rank-1 term),
        # var = ssq - ms^2/C, ri = 1/sqrt(var)
        kst = PP.tile([K, 4], F32, name="kst")  # cols u0, ri0, u1, ri1
        vv = PP.tile([K, 2], F32, name="vv")
        ww = PP.tile([K, 2], F32, name="ww")
        for s in (0, 1):
            nc.vector.tensor_scalar(ww[:, s:s + 1], stats[:, 2 * s:2 * s + 1],
                                    stats[:, 2 * s:2 * s + 1], 1.0 / C,
                                    op0=mult, op1=mult)
            nc.vector.tensor_tensor(vv[:, s:s + 1],
                                    stats[:, 2 * s + 1:2 * s + 2],
                                    ww[:, s:s + 1],
                                    op=mybir.AluOpType.subtract)
            nc.vector.tensor_scalar_mul(kst[:, 2 * s:2 * s + 1],
                                        stats[:, 2 * s:2 * s + 1],
                                        RC if s == 0 else -RC)
        sd = PP.tile([K, 2], F32, name="sd")
        nc.scalar.sqrt(sd[:], vv[:])
        for s in (0, 1):
            nc.vector.reciprocal(kst[:, 2 * s + 1:2 * s + 2], sd[:, s:s + 1])
        # transpose (u0, ri0, u1, ri1) columns to rows of kT [1, 4*K]
        kT = TLP.tile([1, 4 * K], F32, name="kT", tag="tl2")
        for j in range(4):
            nc.tensor.matmul(kT[:, j * K:(j + 1) * K],
                             lhsT=kst[:, j:j + 1], rhs=id_f[:],
                             is_transpose=True,
                             start=(j == 0), stop=(j == 3))
        kT_sb = PP.tile([1, 4 * K], F32, name="kT_sb")
        nc.scalar.copy(kT_sb[:], kT[:])
        # po = G - (C m0) (m1)^T  (u1 pre-negated)
        po = TLP.tile([K, K], F32, name="po", tag="tl3")
        for h in (0, 1):
            nc.tensor.matmul(po[:], lhsT=Xt[:, h * K:(h + 1) * K],
                             rhs=Xt[:, W2 + h * K:W2 + (h + 1) * K],
                             start=(h == 0), stop=False)
        nc.tensor.matmul(po[:], lhsT=kT_sb[:, 0:K], rhs=kT_sb[:, 2 * K:3 * K],
                         start=False, stop=True)
        # broadcast ri1 across partitions, then out = po * ri0 * ri1
        bci = TLP.tile([K, K], F32, name="bci", tag="tl4")
        nc.tensor.matmul(bci[:], lhsT=onesr19[:], rhs=kT_sb[:, 3 * K:4 * K],
                         start=True, stop=True)
        bci_sb = PP.tile([K, K], F32, name="bci_sb")
        nc.vector.tensor_copy(bci_sb[:], bci[:])
        osb = PP.tile([K, K], F32, name="osb")
        nc.vector.scalar_tensor_tensor(osb[:], po[:], kst[:, 1:2], bci_sb[:],
                                       op0=mult, op1=mult)
        nc.sync.dma_start(out_d[:], osb[:])


def build(n_cores=N_CORES):
    nc = bacc.Bacc("TRN2", target_bir_lowering=False, debug=False,
                   enable_asserts=False, num_devices=n_cores)
    pret_d = [nc.dram_tensor(f"pret{s}", [128, NCHUNK * K], BF16,
                             kind="ExternalInput").ap() for s in (1, 2)]
    ftrseg_d = [
        [nc.dram_tensor(f"ftr{s + 1}s{si}", [128, nch * CCW], BF16,
                        kind="ExternalInput").ap()
         for si, nch in enumerate(SEG_SIZES)]
        for s in (0, 1)]
    identf_d = nc.dram_tensor("identf", [K, K], F32, kind="ExternalInput").ap()
    out_d = nc.dram_tensor("out", [K, K], F32, kind="ExternalOutput").ap()
    with tile.TileContext(nc) as tc:
        build_body(nc, tc, pret_d, ftrseg_d, identf_d, out_d, n_cores)
    nc.compile()
    return nc


_NC_CACHE = {}


def _get_nc():
    if "nc" not in _NC_CACHE:
        _NC_CACHE["nc"] = build(N_CORES)
    return _NC_CACHE["nc"]


class Runner:
    """Executes the compiled Bass program on the first `n_cores` jax
    devices via shard_map, with inputs pre-staged on the devices (the
    analog of the native path's input pre-load in run_neff) so all
    cores start the NEFF near-simultaneously."""

    def __init__(self, nc, n_cores):
        import jax
        from jax.experimental.shard_map import shard_map
        from jax.sharding import Mesh, PartitionSpec, NamedSharding

        bass2jax.install_neuronx_cc_hook()
        self.jax = jax
        self.nc = nc
        self.n_cores = n_cores
        assert nc.dbg_addr is None
        partition_name = (nc.partition_id_tensor.name
                          if nc.partition_id_tensor else None)
        in_names, out_names, out_avals = [], [], []
        for alloc in nc.m.functions[0].allocations:
            if not isinstance(alloc, mybir.MemoryLocationSet):
                continue
            name = alloc.memorylocations[0].name
            if alloc.kind == "ExternalInput":
                if name != partition_name:
                    in_names.append(name)
            elif alloc.kind == "ExternalOutput":
                shape = tuple(alloc.tensor_shape)
                dtype = mybir.dt.np(alloc.dtype)
                out_names.append(name)
                out_avals.append(jax.core.ShapedArray(shape, dtype))
        self.param_names = list(in_names)
        n_params = len(in_names)
        full_in_names = list(in_names) + list(out_names)
        if partition_name is not None:
            full_in_names.append(partition_name)
        full_in_names = tuple(full_in_names)
        donate = tuple(range(n_params, n_params + len(out_names)))
        self.out_names = out_names
        self.out_avals = out_avals

        def _body(*args):
            operands = list(args)
            if partition_name is not None:
                operands.append(bass2jax.partition_id_tensor())
            outs = bass2jax._bass_exec_p.bind(
                *operands,
                out_avals=tuple(out_avals),
                in_names=full_in_names,
                out_names=tuple(out_names),
                lowering_input_output_aliases=(),
                sim_require_finite=True,
                sim_require_nnan=True,
                nc=nc,
            )
            return tuple(outs)

        devices = jax.devices()[:n_cores]
        assert len(devices) == n_cores
        self.mesh = Mesh(np.asarray(devices), ("core",))
        in_specs = (PartitionSpec("core"),) * (n_params + len(out_names))
        out_specs = (PartitionSpec("core"),) * len(out_names)
        self.fn = jax.jit(
            shard_map(_body, mesh=self.mesh, in_specs=in_specs,
                      out_specs=out_specs, check_rep=False),
            donate_argnums=donate, keep_unused=True)
        self.sharding = NamedSharding(self.mesh, PartitionSpec("core"))

    def put(self, in_maps):
        concat = [
            np.concatenate([np.asarray(in_maps[c][n])
                            for c in range(self.n_cores)], axis=0)
            for n in self.param_names
        ]
        arrs = [self.jax.device_put(a, self.sharding) for a in concat]
        self.jax.block_until_ready(arrs)
        return arrs

    def zeros(self):
        zs = [self.jax.device_put(
            np.zeros((self.n_cores * a.shape[0], *a.shape[1:]), a.dtype),
            self.sharding) for a in self.out_avals]
        self.jax.block_until_ready(zs)
        return zs

    def exec(self, dev_in):
        outs = self.fn(*dev_in, *self.zeros())
        self.jax.block_until_ready(outs)
        return {
            name: np.asarray(outs[i]).reshape(
                self.n_cores, *self.out_avals[i].shape)
            for i, name in enumerate(self.out_names)
        }


def _get_runner():
    if "runner" not in _NC_CACHE:
        _NC_CACHE["runner"] = Runner(_get_nc(), N_CORES)
    return _NC_CACHE["runner"]


def make_in_maps(preds1, feats1, preds2, feats2):
    import ml_dtypes
    bf16 = ml_dtypes.bfloat16
    identf = np.eye(K, dtype=np.float32)
    per_stream = {}
    for s, (preds, feats) in enumerate(
            ((preds1, feats1), (preds2, feats2)), start=1):
        # preds [B,K,H,W] -> [B, W(v), H(u), K] -> [B, 128, 128*19]:
        # chunk u's columns are P^T[u*128:(u+1)*128, :19] with the
        # spatial index on partitions
        pr = np.ascontiguousarray(
            preds.astype(bf16).transpose(0, 3, 2, 1)
        ).reshape(B, 128, NCHUNK * K)
        # feats [B,C,H,W] -> [B, W, H, C (+ ones)] -> [B, 128, 128*257]:
        # chunk u is the [w, c] block at h=u, matching pret's chunking;
        # the fused ones column makes psum[:, 256] the mask sums
        ft = np.empty((B, W, H, CCW), dtype=bf16)
        ft[..., :C] = feats.astype(bf16).transpose(0, 3, 2, 1)
        ft[..., C] = 1.0
        per_stream[s] = (pr, ft.reshape(B, 128, NCHUNK * CCW))
    in_maps = []
    for b in range(B):
        m = {
            "pret1": per_stream[1][0][b],
            "pret2": per_stream[2][0][b],
            "identf": identf,
        }
        for s in (1, 2):
            base = 0
            for si, nch in enumerate(SEG_SIZES):
                m[f"ftr{s}s{si}"] = np.ascontiguousarray(
                    per_stream[s][1][b][:, base * CCW:(base + nch) * CCW])
                base += nch
        in_maps.append(m)
    return in_maps


def kernel(preds1, feats1, preds2, feats2):
    runner = _get_runner()
    in_maps = make_in_maps(preds1, feats1, preds2, feats2)
    dev_in = runner.put(in_maps)
    outs = runner.exec(dev_in)
    return np.asarray(outs["out"][0], dtype=np.float32)
